# revision 20
# baseline (speedup 1.0000x reference)
"""Trainium2 Bass kernel for coverage-attention (Bahdanau + coverage).

Reference computation (fp32):
    enc   = encoder_outputs.transpose(1,0,2)            # [B,T,H]
    x     = concat([hidden_rep, enc, cov*W_cov], -1)    # [B,T,3H]
    energy= relu(x @ W_attn.T + b_attn)                 # [B,T,H]
    scores= energy @ v                                  # [B,T]
    attn  = softmax(scores, axis=1)
    out   = (attn[:,None,:], coverage + attn)

Decomposition used here (W_attn = [W1 | W2 | W3], each [H,H]):
    z[b,t,h] = (enc @ W2.T)[b,t,h] + a[b,h] + cov[b,t]*u[h]
      a = hidden[0] @ W1.T + b_attn      (tiny, host-precomputed)
      u = W3 @ W_cov[:,0]                (tiny, host-precomputed)
    scores[b,t] = sum_h v[h]*relu(z[b,t,h])

|v[h]| is folded into the h-columns of W2.T / u / a on the host
(relu(|v|*z) == |v|*relu(z)), and the h axis is permuted so all v>=0
columns come first.  Then scores = S_pos - S_neg over the two column
slices.

Per core (batch-parallel over 8 cores, 4 batches each), per [128t,512h]
tile:
  - PE: K=2 matmul ([cov;1].T @ [u;a_b]) seeds PSUM with the coverage
    rank-1 term + bias, then 4 bf16 matmuls [128k,128t].T @ [128k,512h]
    accumulate enc @ W2.T.  100%-utilization MACs; this is the
    bottleneck engine (~68us/core).
  - DVE: one tensor_scalar per sign-slice does relu+sign+reduce in a
    single op: out=(pz max 0)*(+-1), accum_out=sum -> score column.
  - softmax per batch: PE transpose [128,16]->[16,128], gpsimd
    partition reductions, ACT exp, DVE reciprocal/scale.
"""

import os
import sys

import numpy as np

for _p in ("/opt/trn_rl_repo", "/root/.axon_site/_ro/trn_rl_repo"):
    if os.path.isdir(_p) and _p not in sys.path:
        sys.path.insert(0, _p)
        break

import ml_dtypes  # noqa: E402

H = 512
B = 32
T = 2048
N_CORES = 8
BPC = B // N_CORES          # batches per core
TC = T // 128               # 16 score columns (t-tiles) per batch
TCG = 4                     # t-tile groups of 512 t each
KC = 4                      # k chunks of 128

_PROGRAM_CACHE: dict = {}


def _build_program(p_pos: int, reps: int = 1):
    """Build + compile the single-core Bass/Tile program (SPMD across 8).

    reps>1 repeats the whole computation back-to-back inside one NEFF
    (idempotent), for wall-clock benchmarking that cancels host overhead.
    """
    from contextlib import ExitStack

    import concourse.tile as tile
    from concourse import bacc, mybir

    f32 = mybir.dt.float32
    bf16 = mybir.dt.bfloat16
    Alu = mybir.AluOpType
    Act = mybir.ActivationFunctionType

    nc = bacc.Bacc(
        "TRN2",
        target_bir_lowering=False,
        debug=False,
        enable_asserts=False,
        num_devices=N_CORES,
    )

    enc_d = nc.dram_tensor("enc_in", [BPC, H, T], bf16, kind="ExternalInput").ap()
    w2t_d = nc.dram_tensor("w2t_in", [H, H], bf16, kind="ExternalInput").ap()
    covx_d = nc.dram_tensor("covx_in", [2, BPC, T], bf16, kind="ExternalInput").ap()
    rhx_d = nc.dram_tensor("rhx_in", [2, BPC, H], bf16, kind="ExternalInput").ap()
    covt_d = nc.dram_tensor("covt_in", [BPC, TC, 128], f32, kind="ExternalInput").ap()
    idn_d = nc.dram_tensor("iden_in", [128, 128], f32, kind="ExternalInput").ap()
    # fused output: [..., 0:128] = attn, [..., 128:256] = coverage_new
    out_d = nc.dram_tensor("out2_out", [BPC, TC, 256], f32, kind="ExternalOutput").ap()

    with tile.TileContext(nc) as tc, ExitStack() as ctx:
        singles = ctx.enter_context(tc.tile_pool(name="singles", bufs=1))
        encp = ctx.enter_context(tc.tile_pool(name="encp", bufs=4))
        encpb = ctx.enter_context(tc.tile_pool(name="encpb", bufs=8))
        scrapp = ctx.enter_context(tc.tile_pool(name="scrapp", bufs=3))
        scorep = ctx.enter_context(tc.tile_pool(name="scorep", bufs=2))
        smallp = ctx.enter_context(tc.tile_pool(name="smallp", bufs=2))
        outp = ctx.enter_context(tc.tile_pool(name="outp", bufs=2))
        psum = ctx.enter_context(tc.tile_pool(name="psum", bufs=5, space="PSUM"))
        psum_t = ctx.enter_context(tc.tile_pool(name="psum_t", bufs=2, space="PSUM"))

        # --- constants; interleave first enc group so PE starts ASAP ---
        covx_sb = singles.tile([2, BPC, T], bf16)
        nc.sync.dma_start(out=covx_sb[:], in_=covx_d[:])
        rhx_sb = singles.tile([2, BPC, H], bf16)
        nc.sync.dma_start(out=rhx_sb[:], in_=rhx_d[:])
        w2t_sb = singles.tile([128, KC, H], bf16)
        first_group = []
        for kc in range(KC):
            nc.sync.dma_start(out=w2t_sb[:, kc, :], in_=w2t_d[kc * 128:(kc + 1) * 128, :])
            et = encp.tile([128, 512], bf16, tag="enc_t0")
            nc.sync.dma_start(out=et[:], in_=enc_d[0, kc * 128:(kc + 1) * 128, 0:512])
            first_group.append(et)
        # rest of b0 rows in one medium DMA per k-chunk
        b0_rest = []
        for kc in range(KC):
            et = encp.tile([128, 3 * 512], bf16, tag="enc_t1")
            nc.sync.dma_start(out=et[:], in_=enc_d[0, kc * 128:(kc + 1) * 128, 512:T])
            b0_rest.append(et)
        # cold constants (not needed until the first epilogue) on another queue
        covt_sb = singles.tile([TC, BPC, 128], f32)
        nc.scalar.dma_start(
            out=covt_sb[:], in_=covt_d.rearrange("b q j -> q b j")
        )
        idn_sb = singles.tile([128, 128], f32)
        nc.scalar.dma_start(out=idn_sb[:], in_=idn_d[:])

        big_tiles: dict = {}
        for rep in range(reps):
          for b in range(BPC):
            sp = scorep.tile([128, TC], f32, tag="sp")
            sm = scorep.tile([128, TC], f32, tag="sm")
            if p_pos == 0:
                nc.vector.memset(sp[:], 0.0)
            if p_pos == H:
                nc.vector.memset(sm[:], 0.0)
            # prefetch the whole next batch as 4 big DMAs (fixed per-DMA
            # HWDGE descriptor-gen cost dominates; batch to amortize)
            if b + 1 < BPC:
                nxt = []
                for kc in range(KC):
                    et = encpb.tile([128, T], bf16, tag="enc_big")
                    nc.sync.dma_start(
                        out=et[:], in_=enc_d[b + 1, kc * 128:(kc + 1) * 128, :]
                    )
                    nxt.append(et)
                big_tiles[b + 1] = nxt
            for tcg in range(TCG):
                for j in range(4):
                    tci = tcg * 4 + j
                    pz = psum.tile([128, H], f32, tag="pz")
                    # K=2 rank-1 seed: [cov;1].T @ [u_s; a_b]  ->  cov*u + cb
                    nc.tensor.matmul(
                        pz[:],
                        lhsT=covx_sb[:, b, tci * 128:(tci + 1) * 128],
                        rhs=rhx_sb[:, b, :],
                        start=True,
                        stop=False,
                    )
                    for kc in range(KC):
                        if b == 0 and tcg == 0:
                            lhsT = first_group[kc][:, j * 128:(j + 1) * 128]
                        elif b == 0:
                            lhsT = b0_rest[kc][
                                :, (tcg - 1) * 512 + j * 128:(tcg - 1) * 512 + (j + 1) * 128
                            ]
                        else:
                            lhsT = big_tiles[b][kc][
                                :, tcg * 512 + j * 128:tcg * 512 + (j + 1) * 128
                            ]
                        nc.tensor.matmul(
                            pz[:],
                            lhsT=lhsT,
                            rhs=w2t_sb[:, kc, :],
                            start=False,
                            stop=(kc == KC - 1),
                        )
                    # relu + reduce in one DVE op per sign-slice:
                    # out = max(pz, 0); accum_out = reduce(out, op1=add)
                    if p_pos > 0:
                        scr = scrapp.tile([128, H], f32, tag="scr")
                        nc.vector.tensor_scalar(
                            out=scr[:, 0:p_pos],
                            in0=pz[:, 0:p_pos],
                            scalar1=0.0,
                            scalar2=None,
                            op0=Alu.max,
                            op1=Alu.add,
                            accum_out=sp[:, tci:tci + 1],
                        )
                    if p_pos < H:
                        scr2 = scrapp.tile([128, H], f32, tag="scr2")
                        nc.vector.tensor_scalar(
                            out=scr2[:, p_pos:H],
                            in0=pz[:, p_pos:H],
                            scalar1=0.0,
                            scalar2=None,
                            op0=Alu.max,
                            op1=Alu.add,
                            accum_out=sm[:, tci:tci + 1],
                        )

            # ---- per-batch epilogue: softmax over all 2048 t ----
            s_sb = smallp.tile([128, TC], f32, tag="s_sb")
            nc.vector.tensor_sub(s_sb[:], sp[:], sm[:])
            ps_t = psum_t.tile([TC, 128], f32, tag="ps_t")
            nc.tensor.transpose(ps_t[:], s_sb[:], idn_sb[:])
            # scores are O(3), so exp needs no max-subtraction (softmax is
            # shift-invariant; reference only subtracts max for range safety)
            expT = smallp.tile([TC, 128], f32, tag="expT")
            rsum = smallp.tile([TC, 1], f32, tag="rsum")
            nc.scalar.activation(
                out=expT[:], in_=ps_t[:], func=Act.Exp, accum_out=rsum[:]
            )
            zt = smallp.tile([1, 1], f32, tag="zt")
            nc.gpsimd.tensor_reduce(
                out=zt[:], in_=rsum[:], axis=mybir.AxisListType.XYZWC, op=Alu.add
            )
            rz = smallp.tile([1, 1], f32, tag="rz")
            nc.vector.reciprocal(rz[:], zt[:])
            rzb = smallp.tile([TC, 1], f32, tag="rzb")
            nc.gpsimd.partition_broadcast(rzb[:], rz[:])
            o = outp.tile([TC, 256], f32, tag="o")
            nc.vector.tensor_scalar_mul(o[:, 0:128], expT[:], rzb[:])
            nc.vector.tensor_add(o[:, 128:256], o[:, 0:128], covt_sb[:, b, :])
            nc.sync.dma_start(out=out_d[b], in_=o[:])

    nc.compile()
    return nc


def _get_program(p_pos: int, reps: int = 1):
    key = (p_pos, reps)
    if key not in _PROGRAM_CACHE:
        _PROGRAM_CACHE[key] = _build_program(p_pos, reps)
    return _PROGRAM_CACHE[key]


def _prepare(hidden, encoder_outputs, coverage, W_attn, b_attn, v, W_cov):
    """Host-side sharding + weight folding. Returns (p_pos, in_maps)."""
    hidden = np.asarray(hidden, dtype=np.float32)
    encoder_outputs = np.asarray(encoder_outputs, dtype=np.float32)
    coverage = np.asarray(coverage, dtype=np.float32)
    W_attn = np.asarray(W_attn, dtype=np.float32)
    b_attn = np.asarray(b_attn, dtype=np.float32)
    v = np.asarray(v, dtype=np.float32)
    W_cov = np.asarray(W_cov, dtype=np.float32)

    W1 = W_attn[:, :H].astype(np.float64)
    W2 = W_attn[:, H:2 * H].astype(np.float64)
    W3 = W_attn[:, 2 * H:].astype(np.float64)
    u = W3 @ W_cov[:, 0].astype(np.float64)                      # [H]
    a = hidden[0].astype(np.float64) @ W1.T + b_attn.astype(np.float64)  # [B,H]

    order = np.argsort(v < 0, kind="stable")                     # v>=0 first
    p_pos = int((v >= 0).sum())
    vabs = np.abs(v[order].astype(np.float64))

    w2t_s = (W2[order, :].T * vabs[None, :])                     # [k, h']
    w2t_bf = w2t_s.astype(np.float32).astype(ml_dtypes.bfloat16)
    u_s = (u[order] * vabs).astype(np.float32)                   # [H]
    cb_s = (a[:, order] * vabs[None, :]).astype(np.float32)      # [B, H]

    ident = np.eye(128, dtype=np.float32)

    in_maps = []
    for c in range(N_CORES):
        bs = slice(c * BPC, (c + 1) * BPC)
        e = encoder_outputs[:, bs, :]                            # [T, BPC, H]
        enc_bf = np.ascontiguousarray(e.transpose(1, 2, 0)).astype(
            ml_dtypes.bfloat16
        )                                                        # [BPC, H, T]
        cov_c = coverage[bs]                                     # [BPC, T]
        covt = np.ascontiguousarray(cov_c.reshape(BPC, TC, 128))
        covx = np.empty((2, BPC, T), dtype=ml_dtypes.bfloat16)
        covx[0] = cov_c.astype(ml_dtypes.bfloat16)
        covx[1] = np.float32(1.0)
        rhx = np.empty((2, BPC, H), dtype=ml_dtypes.bfloat16)
        rhx[0] = u_s[None, :].astype(ml_dtypes.bfloat16)
        rhx[1] = cb_s[bs].astype(ml_dtypes.bfloat16)
        in_maps.append(
            {
                "enc_in": enc_bf,
                "w2t_in": w2t_bf,
                "covx_in": covx,
                "rhx_in": rhx,
                "covt_in": covt,
                "iden_in": ident,
            }
        )
    return p_pos, in_maps


def _run(inputs: dict, trace: bool = False, reps: int = 1):
    """Run on 8 NeuronCores. Returns ((attn, covnew), BassKernelResults)."""
    from concourse import bass_utils

    p_pos, in_maps = _prepare(**inputs)
    nc = _get_program(p_pos, reps)
    res = bass_utils.run_bass_kernel_spmd(
        nc, in_maps, core_ids=list(range(N_CORES)), trace=trace
    )
    outs = np.concatenate(
        [res.results[c]["out2_out"] for c in range(N_CORES)], axis=0
    ).astype(np.float32)                                        # [B, TC, 256]
    attn = np.ascontiguousarray(outs[:, :, 0:128]).reshape(B, T)
    covn = np.ascontiguousarray(outs[:, :, 128:256]).reshape(B, T)
    return (attn[:, None, :], covn), res


def kernel(hidden, encoder_outputs, coverage, W_attn, b_attn, v, W_cov):
    out, _ = _run(
        dict(
            hidden=hidden,
            encoder_outputs=encoder_outputs,
            coverage=coverage,
            W_attn=W_attn,
            b_attn=b_attn,
            v=v,
            W_cov=W_cov,
        )
    )
    return out
